# revision 4
# baseline (speedup 1.0000x reference)
"""Trainium2 Bass kernel for nn_LocalEnergyCore (sampling / local energy MLP).

Contract: kernel(**inputs) takes FULL unsharded inputs, returns FULL output
(scalar f32). Internally shards z along batch across 8 NeuronCores.

Per-core device program (B_loc = 512 samples):
  - Host pre-gathers each site's 3x3xK neighborhood into a packed fp8 tensor
    ctx [73, 50, 512]: partition r = ctx entry (position-major, bit-minor,
    as in the reference), +1 ones row (folds b1 into the matmul); free dims
    = (site, batch). The dropped center-self entry is handled by zeroing
    that row of the per-variant weight matrix.
  - L1: 25 fp8 DoubleRow matmuls, one per site PAIR. lhsT [73, 2, 128] is
    block-diagonal ([W_a | 0], [0 | W_b]) so psum [128, 512] holds both
    sites' hidden pre-activations (h on partitions 0-63 / 64-127).
  - Relu + fp32->fp8 cast evacuates psum to SBUF [128, 2, 512] tiles,
    alternating between the ACT and DVE engines.
  - L2: 12 fp8 DoubleRow matmuls + 1 regular, with block W2 columns,
    accumulate all 50 logits into ONE [50, 512] psum tile.
  - One DVE scalar_tensor_tensor: ((logit > -b2) != target) summed over
    batch -> counts [50, 1]; ones-matmul reduces over partitions; ACT
    scales by 1/(B*S). Host sums the 8 per-core partial means.
"""

import sys

for _p in ("/opt/trn_rl_repo",):
    if _p not in sys.path:
        sys.path.insert(0, _p)

import numpy as np
import ml_dtypes

B, K, H, W = 4096, 8, 64, 64
S, HID = 50, 64
NPAIR = S // 2
R = 9 * K + 1          # 72 ctx rows + ones row
N_CORES = 8
B_LOC = B // N_CORES

FP8 = ml_dtypes.float8_e4m3

LAST_RESULTS = None  # test harness introspection


def _host_prep(z, W1, b1, W2, b2, b_idx, i_idx, j_idx):
    """Shard + lay out inputs; returns (in_maps, -b2)."""
    z = np.asarray(z, dtype=np.float32)
    W1 = np.asarray(W1, dtype=np.float32)
    b1 = np.asarray(b1, dtype=np.float32)
    W2 = np.asarray(W2, dtype=np.float32)
    b_idx = np.asarray(b_idx).astype(np.int64)
    i_idx = np.asarray(i_idx).astype(np.int64)
    j_idx = np.asarray(j_idx).astype(np.int64)

    di = np.repeat(np.array([-1, 0, 1]), 3)
    dj = np.tile(np.array([-1, 0, 1]), 3)
    ni = (i_idx[:, None] + di[None, :]) % H          # [S, 9]
    nj = (j_idx[:, None] + dj[None, :]) % W          # [S, 9]

    # [B, K, S, 9] -> ctx entries position-major, bit-minor: [B, S, 72]
    neigh = z[:, :, ni, nj]
    ctx_full = np.transpose(neigh, (0, 2, 3, 1)).reshape(B, S, 9 * K)
    ctx8 = ctx_full.astype(FP8)

    # per-variant [73, HID] weight matrix: entry t row = W1[t - (t > drop)],
    # row drop = 0 (the removed center-self entry), row 72 = b1.
    t = np.arange(9 * K)
    WV = np.zeros((K, R, HID), dtype=np.float32)
    for v in range(K):
        drop = 4 * K + v
        src = t - (t > drop)
        WV[v, :9 * K] = W1[np.minimum(src, 9 * K - 2)]
        WV[v, drop] = 0.0
        WV[v, 9 * K] = b1
    WV8 = WV.astype(FP8)

    # L1 stationary: [73, 50, 128]; pair p half q feeds psum partitions
    # q*64:(q+1)*64 with site 2p+q's variant matrix (block-diagonal).
    w1_np = np.zeros((R, S, 2 * HID), dtype=FP8)
    for s in range(S):
        q = s % 2
        w1_np[:, s, q * HID:(q + 1) * HID] = WV8[b_idx[s]]

    W28 = W2.astype(FP8)
    # L2 DoubleRow stationary [128, 24, 128]: group g = pairs (2g, 2g+1) =
    # sites 4g..4g+3; half q covers pair 2g+q. Columns padded to 128 (zeros
    # beyond col 49) because DoubleRow requires col_grp == 0xf.
    w2dr_np = np.zeros((2 * HID, 2 * (NPAIR // 2), 2 * HID), dtype=FP8)
    for g in range(NPAIR // 2):
        for q in range(2):
            w2dr_np[0:HID, 2 * g + q, 4 * g + 2 * q] = W28
            w2dr_np[HID:2 * HID, 2 * g + q, 4 * g + 2 * q + 1] = W28
    # last (odd) pair: regular matmul [128, 50]
    w2last_np = np.zeros((2 * HID, S), dtype=FP8)
    w2last_np[0:HID, S - 2] = W28
    w2last_np[HID:2 * HID, S - 1] = W28

    in_maps = []
    for c in range(N_CORES):
        bs = slice(c * B_LOC, (c + 1) * B_LOC)
        # ctx [73, 50, 512]: [entry, site, batch]
        ctx_c = np.empty((R, S, B_LOC), dtype=FP8)
        ctx_c[:9 * K] = ctx8[bs].transpose(2, 1, 0)
        ctx_c[9 * K] = np.float32(1.0)
        targ_c = np.ascontiguousarray(
            z[bs, b_idx, i_idx, j_idx].T)             # [50, 512] fp32
        in_maps.append({
            "ctx": np.ascontiguousarray(ctx_c),
            "w1": w1_np,
            "w2dr": w2dr_np,
            "w2last": w2last_np,
            "targ": targ_c,
        })
    return in_maps, -float(np.asarray(b2))


def _build_program(neg_b2):
    """Emit the per-core Bass program (identical across cores)."""
    import concourse.bacc as bacc
    import concourse.mybir as mybir
    import concourse.tile as tile

    fp32 = mybir.dt.float32
    fp8 = mybir.dt.float8e4
    DR = mybir.MatmulPerfMode.DoubleRow

    nc = bacc.Bacc("TRN2", target_bir_lowering=False, debug=False,
                   num_devices=N_CORES)

    ctx_d = nc.dram_tensor("ctx", [R, S, B_LOC], fp8, kind="ExternalInput")
    w1_d = nc.dram_tensor("w1", [R, S, 2 * HID], fp8, kind="ExternalInput")
    w2dr_d = nc.dram_tensor("w2dr", [2 * HID, 2 * (NPAIR // 2), 2 * HID], fp8,
                            kind="ExternalInput")
    w2last_d = nc.dram_tensor("w2last", [2 * HID, S], fp8,
                              kind="ExternalInput")
    targ_d = nc.dram_tensor("targ", [S, B_LOC], fp32, kind="ExternalInput")
    outp = nc.dram_tensor("out", [1, 1], fp32, kind="ExternalOutput")

    # ctx DMA chunks (in pairs): small first so L1_0 starts early
    CHUNKS = [2, 3, 4, 7, 9]
    assert sum(CHUNKS) == NPAIR

    with tile.TileContext(nc) as tc:
        with (
            tc.tile_pool(name="const", bufs=1) as cpool,
            tc.tile_pool(name="hsb", bufs=3) as hpool,
            tc.tile_pool(name="ps", bufs=4, space="PSUM") as pspool,
            tc.tile_pool(name="psl", bufs=1, space="PSUM") as pslpool,
            tc.tile_pool(name="pst", bufs=1, space="PSUM") as pstpool,
        ):
            # --- constants / inputs -----------------------------------
            w1_sb = cpool.tile([R, S, 2 * HID], fp8, tag="w1")
            nc.sync.dma_start(out=w1_sb[:, 0:18, :], in_=w1_d[:, 0:18, :])

            ctx_t = []
            off = 0
            for ci, cn in enumerate(CHUNKS):
                ct = cpool.tile([R, 2 * cn, B_LOC], fp8, tag=f"ctx{ci}",
                                name=f"ctx_sb{ci}")
                ctx_t.append((ct, off))
                off += cn

            ct0, _ = ctx_t[0]
            nc.sync.dma_start(out=ct0[:, :, :], in_=ctx_d[:, 0:4, :])

            w2dr_sb = cpool.tile([2 * HID, 2 * (NPAIR // 2), 2 * HID], fp8,
                                 tag="w2dr")
            nc.sync.dma_start(out=w2dr_sb[:, :, :], in_=w2dr_d[:, :, :])
            w2last_sb = cpool.tile([2 * HID, S], fp8, tag="w2last")
            nc.sync.dma_start(out=w2last_sb[:, :], in_=w2last_d[:, :])

            ct1, o1 = ctx_t[1]
            nc.sync.dma_start(out=ct1[:, :, :],
                              in_=ctx_d[:, 2 * o1:2 * (o1 + 3), :])
            ct2, o2 = ctx_t[2]
            nc.sync.dma_start(out=ct2[:, :, :],
                              in_=ctx_d[:, 2 * o2:2 * (o2 + 4), :])
            nc.sync.dma_start(out=w1_sb[:, 18:S, :], in_=w1_d[:, 18:S, :])
            ct3, o3 = ctx_t[3]
            nc.sync.dma_start(out=ct3[:, :, :],
                              in_=ctx_d[:, 2 * o3:2 * (o3 + 7), :])
            ct4, o4 = ctx_t[4]
            nc.sync.dma_start(out=ct4[:, :, :],
                              in_=ctx_d[:, 2 * o4:2 * (o4 + 9), :])
            targ_sb = cpool.tile([S, B_LOC], fp32, tag="targ")
            nc.sync.dma_start(out=targ_sb[:, :], in_=targ_d[:, :])

            ones_sb = cpool.tile([S, 1], fp32, tag="ones")
            nc.vector.memset(ones_sb[:, :], 1.0)

            logit_ps = pslpool.tile([2 * HID, B_LOC], fp32, tag="logit")

            # --- pair pipeline ----------------------------------------
            def pair_rhs(p):
                for ct, o in reversed(ctx_t):
                    if p >= o:
                        return ct[:, 2 * (p - o):2 * (p - o) + 2, :]
                raise AssertionError

            h_sb = {}   # group g -> [128, 2, 512] fp8 tile

            def emit_l1(p):
                h_ps = pspool.tile([2 * HID, B_LOC], fp32, tag="hps",
                                   name=f"hps{p}")
                nc.tensor.matmul(
                    h_ps[:, :],
                    w1_sb[:, 2 * p:2 * p + 2, :],
                    pair_rhs(p),
                    start=True, stop=True, perf_mode=DR)
                g = p // 2
                if g not in h_sb:
                    h_sb[g] = hpool.tile([2 * HID, 2, B_LOC], fp8,
                                         tag="hsb", name=f"hsb{g}")
                eng = nc.scalar if p % 2 == 0 else nc.vector
                if p % 2 == 0:
                    eng.activation(
                        out=h_sb[g][:, 0, :], in_=h_ps[:, :],
                        func=mybir.ActivationFunctionType.Relu,
                        bias=0.0, scale=1.0)
                else:
                    eng.tensor_scalar_max(h_sb[g][:, 1, :], h_ps[:, :], 0.0)

            def emit_l2(g):
                if 2 * g + 1 < NPAIR:
                    nc.tensor.matmul(
                        logit_ps[:, :],
                        w2dr_sb[:, 2 * g:2 * g + 2, :],
                        h_sb[g][:, :, :],
                        start=(g == 0), stop=False, perf_mode=DR)
                else:
                    nc.tensor.matmul(
                        logit_ps[0:S, :],
                        w2last_sb[:, :],
                        h_sb[g][:, 0, :],
                        start=False, stop=True)

            # L2_g emitted after L1_{2g+3} for pipeline slack; psum pool
            # (bufs=4) throttles L1 to stay 4 pairs ahead of evacuation.
            for p in range(NPAIR):
                emit_l1(p)
                if p >= 3 and p % 2 == 1:
                    emit_l2((p - 3) // 2)
            emit_l2((NPAIR - 3) // 2)      # g = 11
            emit_l2(NPAIR // 2)            # final odd pair

            # --- compare + reduce -------------------------------------
            junk = cpool.tile([S, B_LOC], fp32, tag="junk")
            counts = cpool.tile([S, 1], fp32, tag="counts")
            nc.vector.scalar_tensor_tensor(
                out=junk[:, :], in0=logit_ps[0:S, :], scalar=neg_b2,
                in1=targ_sb[:, :],
                op0=mybir.AluOpType.is_gt, op1=mybir.AluOpType.not_equal,
                accum_out=counts[:, :])

            tot_ps = pstpool.tile([1, 1], fp32, tag="tot")
            nc.tensor.matmul(tot_ps[:, :], ones_sb[:, :], counts[:, :],
                             start=True, stop=True)
            res_sb = cpool.tile([1, 1], fp32, tag="res")
            nc.scalar.activation(out=res_sb[:, :], in_=tot_ps[:, :],
                                 func=mybir.ActivationFunctionType.Copy,
                                 scale=1.0 / float(B * S))
            nc.sync.dma_start(out=outp[:, :], in_=res_sb[:, :])

    nc.compile()
    return nc


def kernel(**inputs):
    global LAST_RESULTS
    from concourse.bass_utils import run_bass_kernel_spmd

    in_maps, neg_b2 = _host_prep(
        inputs["z"], inputs["W1"], inputs["b1"], inputs["W2"],
        inputs["b2"], inputs["b_idx"], inputs["i_idx"], inputs["j_idx"])

    nc = _build_program(neg_b2)

    res = run_bass_kernel_spmd(nc, in_maps, list(range(N_CORES)))
    LAST_RESULTS = res
    total = np.float32(0.0)
    for r in res.results:
        total += np.float32(r["out"][0, 0])
    return np.float32(total)


# revision 5
# speedup vs baseline: 1.8625x; 1.8625x over previous
"""Trainium2 Bass kernel for nn_LocalEnergyCore (sampling / local energy MLP).

Contract: kernel(**inputs) takes FULL unsharded inputs, returns FULL output
(scalar f32). Internally shards z along batch across 8 NeuronCores.

Per-core device program (B_loc = 512 samples):
  - Host pre-gathers each site's 3x3xK neighborhood into a packed fp8 tensor
    ctx [73, 50, 512]: partition r = ctx entry (position-major, bit-minor,
    as in the reference), +1 ones row (folds b1 into the matmul); free dims
    = (site, batch). The dropped center-self entry is handled by zeroing
    that row of the per-variant weight matrix.
  - L1: 25 fp8 DoubleRow matmuls, one per site PAIR. lhsT [73, 2, 128] is
    block-diagonal ([W_a | 0], [0 | W_b]) so psum [128, 512] holds both
    sites' hidden pre-activations (h on partitions 0-63 / 64-127).
  - Relu + fp32->fp8 cast evacuates psum to SBUF [128, 2, 512] tiles,
    alternating between the ACT and DVE engines.
  - L2: 12 fp8 DoubleRow matmuls + 1 regular, with block W2 columns,
    accumulate all 50 logits into ONE [50, 512] psum tile.
  - One DVE scalar_tensor_tensor: ((logit > -b2) != target) summed over
    batch -> counts [50, 1]; ones-matmul reduces over partitions; ACT
    scales by 1/(B*S). Host sums the 8 per-core partial means.
"""

import sys

for _p in ("/opt/trn_rl_repo",):
    if _p not in sys.path:
        sys.path.insert(0, _p)

import numpy as np
import ml_dtypes

B, K, H, W = 4096, 8, 64, 64
S, HID = 50, 64
NPAIR = S // 2
R = 9 * K + 1          # 72 ctx rows + ones row
N_CORES = 8
B_LOC = B // N_CORES

FP8 = ml_dtypes.float8_e4m3

LAST_RESULTS = None  # test harness introspection


def _host_prep(z, W1, b1, W2, b2, b_idx, i_idx, j_idx):
    """Shard + lay out inputs; returns (in_maps, -b2)."""
    z = np.asarray(z, dtype=np.float32)
    W1 = np.asarray(W1, dtype=np.float32)
    b1 = np.asarray(b1, dtype=np.float32)
    W2 = np.asarray(W2, dtype=np.float32)
    b_idx = np.asarray(b_idx).astype(np.int64)
    i_idx = np.asarray(i_idx).astype(np.int64)
    j_idx = np.asarray(j_idx).astype(np.int64)

    di = np.repeat(np.array([-1, 0, 1]), 3)
    dj = np.tile(np.array([-1, 0, 1]), 3)
    ni = (i_idx[:, None] + di[None, :]) % H          # [S, 9]
    nj = (j_idx[:, None] + dj[None, :]) % W          # [S, 9]

    # [B, K, S, 9] -> ctx entries position-major, bit-minor: [B, S, 72]
    neigh = z[:, :, ni, nj]
    ctx_full = np.transpose(neigh, (0, 2, 3, 1)).reshape(B, S, 9 * K)
    ctx8 = ctx_full.astype(FP8)

    # per-variant [73, HID] weight matrix: entry t row = W1[t - (t > drop)],
    # row drop = 0 (the removed center-self entry), row 72 = b1.
    t = np.arange(9 * K)
    WV = np.zeros((K, R, HID), dtype=np.float32)
    for v in range(K):
        drop = 4 * K + v
        src = t - (t > drop)
        WV[v, :9 * K] = W1[np.minimum(src, 9 * K - 2)]
        WV[v, drop] = 0.0
        WV[v, 9 * K] = b1
    WV8 = WV.astype(FP8)

    # L1 stationary: [73, 50, 128]; pair p half q feeds psum partitions
    # q*64:(q+1)*64 with site 2p+q's variant matrix (block-diagonal).
    w1_np = np.zeros((R, S, 2 * HID), dtype=FP8)
    for s in range(S):
        q = s % 2
        w1_np[:, s, q * HID:(q + 1) * HID] = WV8[b_idx[s]]

    W28 = W2.astype(FP8)
    # L2 DoubleRow stationary [128, 24, 128]: group g = pairs (2g, 2g+1) =
    # sites 4g..4g+3; half q covers pair 2g+q. Columns padded to 128 (zeros
    # beyond col 49) because DoubleRow requires col_grp == 0xf.
    w2dr_np = np.zeros((2 * HID, 2 * (NPAIR // 2), 2 * HID), dtype=FP8)
    for g in range(NPAIR // 2):
        for q in range(2):
            w2dr_np[0:HID, 2 * g + q, 4 * g + 2 * q] = W28
            w2dr_np[HID:2 * HID, 2 * g + q, 4 * g + 2 * q + 1] = W28
    # last (odd) pair: regular matmul [128, 50]
    w2last_np = np.zeros((2 * HID, S), dtype=FP8)
    w2last_np[0:HID, S - 2] = W28
    w2last_np[HID:2 * HID, S - 1] = W28

    in_maps = []
    for c in range(N_CORES):
        bs = slice(c * B_LOC, (c + 1) * B_LOC)
        # ctx [73, 50, 512]: [entry, site, batch]
        ctx_c = np.empty((R, S, B_LOC), dtype=FP8)
        ctx_c[:9 * K] = ctx8[bs].transpose(2, 1, 0)
        ctx_c[9 * K] = np.float32(1.0)
        targ_c = np.ascontiguousarray(
            z[bs, b_idx, i_idx, j_idx].T)             # [50, 512] fp32
        in_maps.append({
            "ctx": np.ascontiguousarray(ctx_c),
            "w1": w1_np,
            "w2dr": w2dr_np,
            "w2last": w2last_np,
            "targ": targ_c,
        })
    return in_maps, -float(np.asarray(b2))


def _build_program(neg_b2):
    """Emit the per-core Bass program (identical across cores)."""
    import concourse.bacc as bacc
    import concourse.mybir as mybir
    import concourse.tile as tile

    fp32 = mybir.dt.float32
    fp8 = mybir.dt.float8e4
    DR = mybir.MatmulPerfMode.DoubleRow

    nc = bacc.Bacc("TRN2", target_bir_lowering=False, debug=False,
                   num_devices=N_CORES)

    ctx_d = nc.dram_tensor("ctx", [R, S, B_LOC], fp8, kind="ExternalInput")
    w1_d = nc.dram_tensor("w1", [R, S, 2 * HID], fp8, kind="ExternalInput")
    w2dr_d = nc.dram_tensor("w2dr", [2 * HID, 2 * (NPAIR // 2), 2 * HID], fp8,
                            kind="ExternalInput")
    w2last_d = nc.dram_tensor("w2last", [2 * HID, S], fp8,
                              kind="ExternalInput")
    targ_d = nc.dram_tensor("targ", [S, B_LOC], fp32, kind="ExternalInput")
    outp = nc.dram_tensor("out", [1, 1], fp32, kind="ExternalOutput")

    # ctx DMA chunks (in pairs): small first so L1_0 starts early
    CHUNKS = [2, 3, 4, 7, 9]
    assert sum(CHUNKS) == NPAIR

    with tile.TileContext(nc) as tc:
        with (
            tc.tile_pool(name="const", bufs=1) as cpool,
            tc.tile_pool(name="hsb", bufs=3) as hpool,
            tc.tile_pool(name="ps", bufs=4, space="PSUM") as pspool,
            tc.tile_pool(name="psl", bufs=1, space="PSUM") as pslpool,
            tc.tile_pool(name="pst", bufs=1, space="PSUM") as pstpool,
        ):
            # --- constants / inputs -----------------------------------
            w1_sb = cpool.tile([R, S, 2 * HID], fp8, tag="w1")
            nc.gpsimd.dma_start(out=w1_sb[:, 0:18, :], in_=w1_d[:, 0:18, :])

            ctx_t = []
            off = 0
            for ci, cn in enumerate(CHUNKS):
                ct = cpool.tile([R, 2 * cn, B_LOC], fp8, tag=f"ctx{ci}",
                                name=f"ctx_sb{ci}")
                ctx_t.append((ct, off))
                off += cn

            ct0, _ = ctx_t[0]
            nc.gpsimd.dma_start(out=ct0[:, :, :], in_=ctx_d[:, 0:4, :])

            w2dr_sb = cpool.tile([2 * HID, 2 * (NPAIR // 2), 2 * HID], fp8,
                                 tag="w2dr")
            nc.gpsimd.dma_start(out=w2dr_sb[:, :, :], in_=w2dr_d[:, :, :])
            w2last_sb = cpool.tile([2 * HID, S], fp8, tag="w2last")
            nc.sync.dma_start(out=w2last_sb[:, :], in_=w2last_d[:, :])

            ct1, o1 = ctx_t[1]
            nc.gpsimd.dma_start(out=ct1[:, :, :],
                                in_=ctx_d[:, 2 * o1:2 * (o1 + 3), :])
            ct2, o2 = ctx_t[2]
            nc.gpsimd.dma_start(out=ct2[:, :, :],
                                in_=ctx_d[:, 2 * o2:2 * (o2 + 4), :])
            nc.gpsimd.dma_start(out=w1_sb[:, 18:S, :], in_=w1_d[:, 18:S, :])
            ct3, o3 = ctx_t[3]
            nc.gpsimd.dma_start(out=ct3[:, :, :],
                                in_=ctx_d[:, 2 * o3:2 * (o3 + 7), :])
            ct4, o4 = ctx_t[4]
            nc.gpsimd.dma_start(out=ct4[:, :, :],
                                in_=ctx_d[:, 2 * o4:2 * (o4 + 9), :])
            targ_sb = cpool.tile([S, B_LOC], fp32, tag="targ")
            nc.sync.dma_start(out=targ_sb[:, :], in_=targ_d[:, :])

            ones_sb = cpool.tile([S, 1], fp32, tag="ones")
            nc.vector.memset(ones_sb[:, :], 1.0)

            logit_ps = pslpool.tile([2 * HID, B_LOC], fp32, tag="logit")

            # --- pair pipeline ----------------------------------------
            def pair_rhs(p):
                for ct, o in reversed(ctx_t):
                    if p >= o:
                        return ct[:, 2 * (p - o):2 * (p - o) + 2, :]
                raise AssertionError

            h_sb = {}   # group g -> [128, 2, 512] fp8 tile

            def emit_l1(p):
                h_ps = pspool.tile([2 * HID, B_LOC], fp32, tag="hps",
                                   name=f"hps{p}")
                nc.tensor.matmul(
                    h_ps[:, :],
                    w1_sb[:, 2 * p:2 * p + 2, :],
                    pair_rhs(p),
                    start=True, stop=True, perf_mode=DR)
                g = p // 2
                if g not in h_sb:
                    h_sb[g] = hpool.tile([2 * HID, 2, B_LOC], fp8,
                                         tag="hsb", name=f"hsb{g}")
                eng = nc.scalar if p % 2 == 0 else nc.vector
                if p % 2 == 0:
                    eng.activation(
                        out=h_sb[g][:, 0, :], in_=h_ps[:, :],
                        func=mybir.ActivationFunctionType.Relu,
                        bias=0.0, scale=1.0)
                else:
                    eng.tensor_scalar_max(h_sb[g][:, 1, :], h_ps[:, :], 0.0)

            def emit_l2(g):
                if 2 * g + 1 < NPAIR:
                    nc.tensor.matmul(
                        logit_ps[:, :],
                        w2dr_sb[:, 2 * g:2 * g + 2, :],
                        h_sb[g][:, :, :],
                        start=(g == 0), stop=False, perf_mode=DR)
                else:
                    nc.tensor.matmul(
                        logit_ps[0:S, :],
                        w2last_sb[:, :],
                        h_sb[g][:, 0, :],
                        start=False, stop=True)

            # L2_g emitted after L1_{2g+3} for pipeline slack; psum pool
            # (bufs=4) throttles L1 to stay 4 pairs ahead of evacuation.
            for p in range(NPAIR):
                emit_l1(p)
                if p >= 3 and p % 2 == 1:
                    emit_l2((p - 3) // 2)
            emit_l2((NPAIR - 3) // 2)      # g = 11
            emit_l2(NPAIR // 2)            # final odd pair

            # --- compare + reduce -------------------------------------
            junk = cpool.tile([S, B_LOC], fp32, tag="junk")
            counts = cpool.tile([S, 1], fp32, tag="counts")
            nc.vector.scalar_tensor_tensor(
                out=junk[:, :], in0=logit_ps[0:S, :], scalar=neg_b2,
                in1=targ_sb[:, :],
                op0=mybir.AluOpType.is_gt, op1=mybir.AluOpType.not_equal,
                accum_out=counts[:, :])

            tot_ps = pstpool.tile([1, 1], fp32, tag="tot")
            nc.tensor.matmul(tot_ps[:, :], ones_sb[:, :], counts[:, :],
                             start=True, stop=True)
            res_sb = cpool.tile([1, 1], fp32, tag="res")
            nc.scalar.activation(out=res_sb[:, :], in_=tot_ps[:, :],
                                 func=mybir.ActivationFunctionType.Copy,
                                 scale=1.0 / float(B * S))
            nc.sync.dma_start(out=outp[:, :], in_=res_sb[:, :])

    nc.compile()
    return nc


def kernel(**inputs):
    global LAST_RESULTS
    from concourse.bass_utils import run_bass_kernel_spmd

    in_maps, neg_b2 = _host_prep(
        inputs["z"], inputs["W1"], inputs["b1"], inputs["W2"],
        inputs["b2"], inputs["b_idx"], inputs["i_idx"], inputs["j_idx"])

    nc = _build_program(neg_b2)

    res = run_bass_kernel_spmd(nc, in_maps, list(range(N_CORES)))
    LAST_RESULTS = res
    total = np.float32(0.0)
    for r in res.results:
        total += np.float32(r["out"][0, 0])
    return np.float32(total)
